# revision 2
# baseline (speedup 1.0000x reference)
"""GCN (2-layer + pvt projection) Trainium2 kernel, 8-core SPMD. v5.

v4 + quarter-chunk AllGather pipelining: the packed table is laid out
(half, quarter)-chunk-major so each AllGather moves one contiguous 1.6MB
quarter, issued as soon as the producing 25 tiles are consumed. Under DMA
contention a chunk transfer then completes within its 240-570us of cover,
removing v4's 300-700us Pool-queue stalls on half-table AllGathers.
Gather buckets and windows are unchanged from v4 (gathers span the full
half; idx encodes quarter*12800 + row). x arrives pre-transposed.
"""

import sys

sys.path.insert(0, "/opt/trn_rl_repo")

import numpy as np
import ml_dtypes

from concourse import bass, bacc, mybir, tile
from concourse import bass_utils
from concourse.bass_utils import run_bass_kernel_spmd

# ---- NTFF profiling hook (normally injected by the launcher) -------------


def _install_ntff_hook():
    import types
    import ctypes
    import contextlib

    if "antenv.axon_hooks" in sys.modules:
        return
    hook = None
    so_path = "/opt/axon/libaxon_pjrt.so"
    try:
        lib = ctypes.CDLL(so_path)
        if hasattr(lib, "axon_start_nrt_profile"):
            lib.axon_start_nrt_profile.argtypes = [
                ctypes.POINTER(ctypes.c_int64), ctypes.c_size_t]
            lib.axon_start_nrt_profile.restype = ctypes.c_int64
            lib.axon_stop_nrt_profile.argtypes = [ctypes.c_char_p]
            lib.axon_stop_nrt_profile.restype = ctypes.c_int64

            @contextlib.contextmanager
            def _hook(output_dir, device_ids):
                import jax
                jax.devices()
                if device_ids:
                    ids = (ctypes.c_int64 * len(device_ids))(*device_ids)
                    rc = lib.axon_start_nrt_profile(ids, len(device_ids))
                else:
                    rc = lib.axon_start_nrt_profile(None, 0)
                if rc != 0:
                    raise RuntimeError(f"axon_start_nrt_profile rc={rc}")
                try:
                    yield
                finally:
                    n = lib.axon_stop_nrt_profile(str(output_dir).encode())
                    print(f"ntff profile: {n} file(s) -> {output_dir}")

            hook = _hook
    except OSError:
        pass
    mod = types.ModuleType("antenv.axon_hooks")
    mod.get_axon_ntff_profile_hook = lambda: hook
    mod.set_axon_ntff_profile_hook = lambda h: None
    sys.modules["antenv.axon_hooks"] = mod


_install_ntff_hook()
bass_utils.upload_artifacts = lambda tmpdir: f"local://{tmpdir}"

BF16 = ml_dtypes.bfloat16
NCORES = 8
P = 128
N_LOC = 12800          # local nodes per core (100 tiles)
N_TILES = N_LOC // P   # 100
N_PAD = N_LOC * NCORES  # 102400
LP_LOC = N_LOC // 2    # 6400 local pair rows
LP_HALF = LP_LOC // 2  # 3200 pair rows per half
HROWS = LP_HALF * NCORES  # 25600 table rows per half

FULL = dict(N=100_000, NFEAT=512, NHID=64, NCLASS=40)


# --------------------------------------------------------------------------
# host-side planning
# --------------------------------------------------------------------------

QROWS = HROWS // 2       # 12800 table rows per (half, quarter) chunk
LP_Q = LP_HALF // 2      # 1600 local pair rows per chunk


def _src_map(c):
    """Global src node -> (half, table_row-within-half, parity).

    Table rows are (half, quarter)-chunk-major: within half h, row
    q*QROWS + core*LP_Q + r, so each AllGather quarter-chunk is a
    contiguous output range. Gathers still span the whole half.
    """
    core_c = c // N_LOC
    lc = c % N_LOC
    t_src = lc >> 7
    j = lc & 127
    p = j >> 6
    lp = t_src * 64 + (j & 63)
    h = (lp >= LP_HALF).astype(np.int64)
    lph = lp - h * LP_HALF
    q = lph // LP_Q
    tabrow = q * QROWS + core_c * LP_Q + (lph - q * LP_Q)
    return h, tabrow, p


class Plan:
    """(tile, half, parity)-bucketed slot/window layout, core-uniform.

    Windows per bucket: floor(max_e/128) full 128-slot aligned windows plus
    packed tail windows (shared 128-slot columns across the group's tiles).
    v3: per-tile window lists split by half h; matmul rhs bands described by
    per-window-instance (pos, val) columns instead of expanded lhs.
    """

    def __init__(self, rows, cols, vals, G_T):
        n_groups = N_TILES // G_T
        assert N_TILES % G_T == 0
        self.G_T = G_T
        self.n_groups = n_groups
        self.groups = [list(range(g * G_T, (g + 1) * G_T))
                       for g in range(n_groups)]
        bucket_order = [(0, 0), (0, 1), (1, 0), (1, 1)]
        self.bucket_order = bucket_order

        core = rows // N_LOC
        per = []
        counts = np.zeros((NCORES, N_TILES, 2, 2), np.int64)
        border = np.zeros((2, 2), np.int64)
        for i, (h, p) in enumerate(bucket_order):
            border[h, p] = i
        for k in range(NCORES):
            m = core == k
            d = (rows[m] - k * N_LOC).astype(np.int64)
            c = cols[m].astype(np.int64)
            v = vals[m].astype(np.float32)
            t = d >> 7
            h, tr, p = _src_map(c)
            o = np.lexsort((d, border[h, p], t))
            t, h, p, tr, d, v = t[o], h[o], p[o], tr[o], d[o], v[o]
            np.add.at(counts[k], (t, h, p), 1)
            per.append((t, h, p, tr, d, v))

        maxe = counts.max(axis=0)  # [t, h, p]
        # every (tile, half) needs >=1 window so each phase's psum is valid
        for hh in range(2):
            empty_h = maxe[:, hh, :].sum(axis=1) == 0
            maxe[empty_h, hh, 0] = 1
        nfull = maxe // P
        nrem = maxe % P

        # ---- region layout: bulk windows then packed tail columns ----
        slot_base = np.zeros((N_TILES, 2, 2), np.int64)
        tb1 = np.zeros((N_TILES, 2, 2), np.int64)
        tk1 = np.zeros((N_TILES, 2, 2), np.int64)
        tb2 = np.zeros((N_TILES, 2, 2), np.int64)
        win_base = np.zeros((N_TILES, 2, 2), np.int64)
        ntail = np.zeros((N_TILES, 2, 2), np.int64)
        self.region_off = {}   # (g,h,p) -> (slot_off, n_slots)
        off = 0
        nwin_total = 0
        for g in range(n_groups):
            for h, p in bucket_order:
                b = off
                for t in self.groups[g]:
                    slot_base[t, h, p] = off
                    off += nfull[t, h, p] * P
                fill = P
                for t in self.groups[g]:
                    r = int(nrem[t, h, p])
                    if r == 0:
                        continue
                    if fill == P:
                        col_start = off
                        off += P
                        fill = 0
                    k1 = min(r, P - fill)
                    tb1[t, h, p] = col_start + fill
                    tk1[t, h, p] = k1
                    ntail[t, h, p] = 1
                    fill += k1
                    if fill == P and k1 < r:
                        col_start = off
                        off += P
                        fill = r - k1
                        tb2[t, h, p] = col_start
                        ntail[t, h, p] = 2
                self.region_off[(g, h, p)] = (b, off - b)
                for t in self.groups[g]:
                    win_base[t, h, p] = nwin_total
                    nwin_total += int(nfull[t, h, p] + ntail[t, h, p])
        S = off
        self.S = S
        self.nwin_total = nwin_total

        # ---- per-core slot/window assignment ----
        idx_s = np.zeros((NCORES, S), np.int16)
        val_s = np.zeros((NCORES, S), np.float32)
        pos_s = np.zeros((NCORES, S), np.int64)
        valid = np.zeros((NCORES, S), bool)
        win_of = np.zeros((NCORES, S), np.int64)
        for k in range(NCORES):
            t, h, p, tr, d, v = per[k]
            key = ((t * 2 + h) * 2 + p)
            ne = len(key)
            if ne:
                starts = np.r_[0, np.nonzero(np.diff(key))[0] + 1]
                run_id = np.zeros(ne, np.int64)
                run_id[starts[1:]] = 1
                run_id = np.cumsum(run_id)
                rank = np.arange(ne) - starts[run_id]
                nb = nfull[t, h, p] * P
                rt = rank - nb
                s = np.where(
                    rank < nb, slot_base[t, h, p] + rank,
                    np.where(rt < tk1[t, h, p], tb1[t, h, p] + rt,
                             tb2[t, h, p] + rt - tk1[t, h, p]))
                w = win_base[t, h, p] + np.where(
                    rank < nb, rank // P,
                    np.where(rt < tk1[t, h, p], nfull[t, h, p],
                             nfull[t, h, p] + 1))
                idx_s[k, s] = tr.astype(np.int16)
                val_s[k, s] = v
                pos_s[k, s] = d & 127
                valid[k, s] = True
                win_of[k, s] = w

        # ---- per-window spans (union over cores) ----
        lo = np.full(nwin_total, P, np.int64)
        hi = np.zeros(nwin_total, np.int64)
        for k in range(NCORES):
            m = valid[k]
            np.minimum.at(lo, win_of[k, m], pos_s[k, m])
            np.maximum.at(hi, win_of[k, m], pos_s[k, m] + 1)
        none = hi == 0
        lo[none] = 0
        hi[none] = 1

        # ---- per-(tile, half) window instance lists ----
        # instance tuple: (p, colF, M, lo, wid)
        cis_th = {}
        for g in range(n_groups):
            for t in self.groups[g]:
                for h in range(2):
                    cis = []
                    for p in range(2):
                        ro = self.region_off[(g, h, p)][0]
                        for c in range(nfull[t, h, p]):
                            cis.append(
                                (p, (slot_base[t, h, p] + c * P - ro) // P,
                                 win_base[t, h, p] + c))
                        if ntail[t, h, p] >= 1:
                            cis.append((p, (tb1[t, h, p] - ro) // P,
                                        win_base[t, h, p] + nfull[t, h, p]))
                        if ntail[t, h, p] == 2:
                            cis.append((p, (tb2[t, h, p] - ro) // P,
                                        win_base[t, h, p]
                                        + nfull[t, h, p] + 1))
                    assert cis, f"tile {t} half {h} has no windows"
                    # first window of each half full-span (psum start zeroes
                    # the whole 128-dest row)
                    lo[cis[0][2]], hi[cis[0][2]] = 0, P
                    cis_th[(t, h)] = cis

        # lhs band column offsets, in (g, h)-major phase processing order
        self.tile_windows = {}       # (t,h) -> list of (p,colF,M,lo,col)
        self.gh_span = {}            # (g,h) -> (col0, ncols)
        lhs_off_w = np.zeros(nwin_total, np.int64)
        col = 0
        for g in range(n_groups):
            for h in range(2):
                c0 = col
                for t in self.groups[g]:
                    lst = []
                    for (p, colF, wid) in cis_th[(t, h)]:
                        M = int(hi[wid] - lo[wid])
                        lst.append((p, int(colF), M, int(lo[wid]), col))
                        lhs_off_w[wid] = col
                        col += M
                    self.tile_windows[(t, h)] = lst
                self.gh_span[(g, h)] = (c0, col - c0)
        self.L = col

        # per-core lhs band matrix [128, L]
        self.lhs_np = []
        for k in range(NCORES):
            m = valid[k]
            sl = np.nonzero(m)[0]
            lhs = np.zeros((P, self.L), np.float32)
            rowi = sl % P
            wids = win_of[k, sl]
            coli = lhs_off_w[wids] + pos_s[k, sl] - lo[wids]
            assert (coli >= 0).all() and (coli < self.L).all()
            lhs[rowi, coli] = val_s[k, sl]
            self.lhs_np.append(lhs.astype(BF16))

            # (keep idx packing identical to v2)
        self.idx_np = []
        for k in range(NCORES):
            idx16 = np.zeros((16, S // 16), np.int16)
            ss = np.arange(S)
            idx16[ss % 16, ss // 16] = idx_s[k]
            self.idx_np.append(np.tile(idx16, (NCORES, 1)))

        self.cmax = int(max(n // P for (_, n) in self.region_off.values()))
        self.lghmax = int(max(n for (_, n) in self.gh_span.values()))


# --------------------------------------------------------------------------
# numpy emulation of the device dataflow (fast host-side correctness check)
# --------------------------------------------------------------------------

def emulate(plan, tab_full, k):
    """tab_full: [2*HROWS, 128] float32 packed table (both halves).
    Returns [N_TILES, 64, 128] result of spmm for core k."""
    out = np.zeros((N_TILES, 64, P), np.float32)
    idx16 = plan.idx_np[k][:16]
    ss = np.arange(plan.S)
    idx_flat = idx16[ss % 16, ss // 16].astype(np.int64)
    lhs = plan.lhs_np[k].astype(np.float32)
    for h in range(2):
        for g in range(plan.n_groups):
            fbs = {}
            for p in range(2):
                soff, n = plan.region_off[(g, h, p)]
                rws = idx_flat[soff:soff + n] + h * HROWS
                fbs[p] = tab_full[rws].reshape(n // P, P, P)
            for t in plan.groups[g]:
                acc = np.zeros((64, P), np.float32)
                for (p, colF, M, lo, co) in plan.tile_windows[(t, h)]:
                    fb = fbs[p][colF][:, p * 64:p * 64 + 64]  # [128, 64]
                    band = lhs[:, co:co + M]
                    acc[:, lo:lo + M] += fb.T @ band
                out[t] += acc
    return out


def pack_table(feat):
    """feat: [N_PAD, 64] -> packed [2*HROWS, 128] chunk-major table."""
    f = feat.reshape(NCORES, N_TILES, 2, 64, 64)
    pr = np.concatenate((f[:, :, 0], f[:, :, 1]), axis=3)
    # [core, h, q, r, :] -> [h, q, core, r, :]
    pr = pr.reshape(NCORES, 2, 2, LP_Q, P).transpose(1, 2, 0, 3, 4)
    return pr.reshape(-1, P)


# --------------------------------------------------------------------------
# device kernel builder
# --------------------------------------------------------------------------

def build_kernel(ep, pp):
    NFEAT, NHID, NCLASS = FULL["NFEAT"], FULL["NHID"], FULL["NCLASS"]
    ncc = NFEAT // P
    f32 = mybir.dt.float32
    bf16 = mybir.dt.bfloat16
    i16 = mybir.dt.int16

    nc = bacc.Bacc("TRN2", target_bir_lowering=False, debug=False,
                   enable_asserts=False, num_devices=NCORES,
                   num_swdge_queues=2)

    x_d = nc.dram_tensor("xt", [NFEAT, N_LOC], bf16, kind="ExternalInput")
    w1_d = nc.dram_tensor("w1", [NFEAT, NHID], bf16, kind="ExternalInput")
    w2_d = nc.dram_tensor("w2", [NHID, NCLASS], bf16, kind="ExternalInput")
    b1_d = nc.dram_tensor("b1", [NHID, 1], f32, kind="ExternalInput")
    b2_d = nc.dram_tensor("b2r", [1, NCLASS], bf16, kind="ExternalInput")
    s_d = nc.dram_tensor("srow", [1, N_LOC], bf16, kind="ExternalInput")
    eidx_d = nc.dram_tensor("eidx", [P, ep.S // 16], i16, kind="ExternalInput")
    elhs_d = nc.dram_tensor("elhs", [P, ep.L], bf16, kind="ExternalInput")
    pidx_d = nc.dram_tensor("pidx", [P, pp.S // 16], i16, kind="ExternalInput")
    plhs_d = nc.dram_tensor("plhs", [P, pp.L], bf16, kind="ExternalInput")
    out_d = nc.dram_tensor("out", [N_LOC, NCLASS], f32, kind="ExternalOutput")

    rg = [list(range(NCORES))]
    qrr = [0]

    def qnext():
        q = qrr[0]
        qrr[0] = (q + 1) % 2
        return q

    with tile.TileContext(nc) as tc:
        with (
            tc.tile_pool(name="dram", bufs=1, space="DRAM") as dram,
            tc.tile_pool(name="const", bufs=1) as cpool,
            tc.tile_pool(name="xt", bufs=8) as xtp,
            tc.tile_pool(name="fbuf", bufs=6) as fpool,
            tc.tile_pool(name="lhsb", bufs=2) as lpool,
            tc.tile_pool(name="stg", bufs=4) as spool,
            tc.tile_pool(name="tmp", bufs=4) as tpool,
            tc.tile_pool(name="sfx", bufs=4) as fxpool,
            tc.tile_pool(name="psS", bufs=4, space="PSUM") as pspool,
            tc.tile_pool(name="psP", bufs=2, space="PSUM") as pkpool,
            tc.tile_pool(name="psO", bufs=2, space="PSUM") as ps2pool,
        ):
            ag1_in = dram.tile([LP_LOC, P], bf16, tag="ag1_in")
            ag2_in = dram.tile([LP_LOC, P], bf16, tag="ag2_in")
            ag3_in = dram.tile([LP_LOC, P], bf16, tag="ag3_in")
            tab1 = nc.dram_tensor("tab1", [2 * HROWS, P], bf16,
                                  kind="Internal", addr_space="Shared")
            tab2 = nc.dram_tensor("tab2", [2 * HROWS, P], bf16,
                                  kind="Internal", addr_space="Shared")
            tab3 = nc.dram_tensor("tab3", [2 * HROWS, P], bf16,
                                  kind="Internal", addr_space="Shared")

            # ---- constants ----
            w1_sb = cpool.tile([P, ncc, NHID], bf16, tag="w1")
            nc.sync.dma_start(
                out=w1_sb[:],
                in_=w1_d.ap().rearrange("(c p) f -> p c f", p=P))
            w2_sb = cpool.tile([NHID, NCLASS], bf16, tag="w2")
            nc.sync.dma_start(out=w2_sb[:], in_=w2_d.ap())
            b1_sb = cpool.tile([NHID, 1], f32, tag="b1")
            nc.sync.dma_start(out=b1_sb[:], in_=b1_d.ap())
            b2_sb = cpool.tile([1, NCLASS], bf16, tag="b2")
            nc.sync.dma_start(out=b2_sb[:], in_=b2_d.ap())
            s_sb = cpool.tile([1, N_LOC], bf16, tag="srow")
            nc.sync.dma_start(out=s_sb[:], in_=s_d.ap())
            ident = cpool.tile([64, 64], bf16, tag="ident")
            from concourse.masks import make_identity
            make_identity(nc, ident[:])
            eidx_sb = cpool.tile([P, ep.S // 16], i16, tag="eidx")
            nc.sync.dma_start(out=eidx_sb[:], in_=eidx_d.ap())
            pidx_sb = cpool.tile([P, pp.S // 16], i16, tag="pidx")
            nc.sync.dma_start(out=pidx_sb[:], in_=pidx_d.ap())

            # SBUF f32 accumulator for cross-phase partial sums
            acc_sb = cpool.tile([NHID, N_LOC], bf16, tag="acc")

            def ag_chunk(ag_in, tab, h, q):
                i0 = h * LP_HALF + q * LP_Q
                o0 = h * HROWS + q * QROWS
                nc.gpsimd.collective_compute(
                    "AllGather", mybir.AluOpType.bypass, replica_groups=rg,
                    ins=[ag_in[i0:i0 + LP_Q, :].opt()],
                    outs=[tab.ap()[o0:o0 + QROWS, :].opt()])

            def stage_packed(ag_in, t, pk):
                stg = spool.tile([64, P], bf16, tag="stg")
                nc.vector.tensor_copy(out=stg[:], in_=pk[:])
                nc.sync.dma_start(
                    out=ag_in[t * 64:(t + 1) * 64, :], in_=stg[:])

            # ---- stage A: XW1 packed table (10 waves of 10 tiles) ----
            WV = 10
            for w in range(N_TILES // WV):
                trange = range(w * WV, (w + 1) * WV)
                r0 = w * WV * P
                xts = []
                for cc in range(ncc):
                    xt = xtp.tile([P, WV * P], bf16, tag="xt")
                    nc.sync.dma_start(
                        out=xt[:],
                        in_=x_d.ap()[cc * P:(cc + 1) * P, r0:r0 + WV * P])
                    xts.append(xt)
                for t in trange:
                    pk = pspool.tile([64, P], f32, tag="ps")
                    for half in range(2):
                        c0 = t * P + half * 64 - r0
                        for cc in range(ncc):
                            nc.tensor.matmul(
                                out=pk[:, half * 64:half * 64 + 64],
                                lhsT=xts[cc][:, c0:c0 + 64],
                                rhs=w1_sb[:, cc, :],
                                start=(cc == 0), stop=(cc == ncc - 1))
                    stage_packed(ag1_in, t, pk)
                if w == 2:
                    ag_chunk(ag1_in, tab1, 0, 0)
                elif w == 4:
                    ag_chunk(ag1_in, tab1, 0, 1)
                elif w == 7:
                    ag_chunk(ag1_in, tab1, 1, 0)
                elif w == 9:
                    ag_chunk(ag1_in, tab1, 1, 1)

            # ---- generic phase-major SpMM pass ----
            # halves: processing order of h phases
            # consume(t, ps, acc_ap) called at the end of the LAST phase
            # ag_after: {(h_idx, gi): fn} fired after group gi of phase h_idx
            def spmm(plan, tab, idx_sb, lhs_d, consume, halves=(0, 1),
                     ag_after=None, group_order=None):
                if group_order is None:
                    group_order = list(range(plan.n_groups))
                ag_after = ag_after or {}

                def gather(g, h, p):
                    soff, n_idx = plan.region_off[(g, h, p)]
                    fb = fpool.tile([P, plan.cmax, P], bf16, tag="F")
                    nc.gpsimd.dma_gather(
                        fb[:, 0:n_idx // P, :],
                        tab.ap()[h * HROWS:(h + 1) * HROWS, :],
                        idx_sb[:, soff // 16:(soff + n_idx) // 16],
                        n_idx, n_idx, P, elem_step=P,
                        single_packet=False, queue_num=qnext())
                    return fb

                for phase, h in enumerate(halves):
                    for gi, g in enumerate(group_order):
                        fbs = {p: gather(g, h, p) for p in range(2)}
                        o0, lg = plan.gh_span[(g, h)]
                        lsb = lpool.tile([P, plan.lghmax], bf16, tag="lhs")
                        nc.sync.dma_start(out=lsb[:, 0:lg],
                                          in_=lhs_d.ap()[:, o0:o0 + lg])
                        for t in plan.groups[g]:
                            wins = plan.tile_windows[(t, h)]
                            ps = pspool.tile([64, P], f32, tag="ps")
                            prev = None
                            for i, (p, colF, M, lo, co) in enumerate(wins):
                                mm = nc.tensor.matmul(
                                    out=ps[:, lo:lo + M],
                                    lhsT=fbs[p][:, colF,
                                                p * 64:p * 64 + 64],
                                    rhs=lsb[:, co - o0:co - o0 + M],
                                    start=(i == 0),
                                    stop=(i == len(wins) - 1))
                                if prev is not None:
                                    tile.add_dep_helper(mm.ins, prev.ins,
                                                        sync=False,
                                                        reason="acc order")
                                prev = mm
                            a = acc_sb[:, t * P:(t + 1) * P]
                            if phase == 0:
                                nc.vector.tensor_copy(out=a, in_=ps[:])
                            else:
                                consume(t, ps, a)
                        fn = ag_after.get((phase, gi))
                        if fn:
                            fn()

            def pack_consume(ag_in, act):
                def f(t, ps, a):
                    hT = act(t, ps, a)
                    pk = pkpool.tile([64, P], bf16, tag="pk")
                    nc.tensor.transpose(out=pk[:, 0:64], in_=hT[:, 0:64],
                                        identity=ident[:])
                    nc.tensor.transpose(out=pk[:, 64:128], in_=hT[:, 64:128],
                                        identity=ident[:])
                    stage_packed(ag_in, t, pk)
                return f

            # ---- gc1: h1 = relu(spmm(adj, XW1) + b1) ----
            def gc1_act(t, ps, a):
                tmpf = tpool.tile([NHID, P], f32, tag="tmp")
                nc.vector.tensor_tensor(out=tmpf[:], in0=ps[:], in1=a,
                                        op=mybir.AluOpType.add)
                hT = spool.tile([NHID, P], bf16, tag="hT")
                nc.scalar.activation(
                    out=hT[:], in_=tmpf[:],
                    func=mybir.ActivationFunctionType.Relu,
                    bias=b1_sb[:, 0:1], scale=1.0)
                return hT

            # gc1 phase-1 forward: chunk (h,q) done after tiles (2h+q+1)*25
            ng = ep.n_groups
            gpc = -(-25 // ep.G_T)  # groups per 25-tile chunk
            chunks_fwd = [(0, 0), (0, 1), (1, 0), (1, 1)]
            ag2s = {(1, min(gpc * (i + 1), ng) - 1):
                    (lambda hq: lambda: ag_chunk(ag2_in, tab2, *hq))(
                        chunks_fwd[i]) for i in range(4)}
            spmm(ep, tab1, eidx_sb, elhs_d, pack_consume(ag2_in, gc1_act),
                 halves=(0, 1), ag_after=ag2s)

            # ---- gc2: A2 = spmm(adj, h1) (no bias; W2 deferred) ----
            # consume in reverse tile order so tab3's h1 half lands early
            def gc2_act(t, ps, a):
                hT = spool.tile([NHID, P], bf16, tag="hT")
                nc.vector.tensor_tensor(out=hT[:], in0=ps[:], in1=a,
                                        op=mybir.AluOpType.add)
                return hT

            # gc2 phase-1 reverse: tiles 99-75 first -> chunk (1,1) first
            rev = list(range(ng - 1, -1, -1))
            chunks_rev = [(1, 1), (1, 0), (0, 1), (0, 0)]
            ag3s = {(1, min(gpc * (i + 1), ng) - 1):
                    (lambda hq: lambda: ag_chunk(ag3_in, tab3, *hq))(
                        chunks_rev[i]) for i in range(4)}
            spmm(ep, tab2, eidx_sb, elhs_d, pack_consume(ag3_in, gc2_act),
                 halves=(0, 1), ag_after=ag3s, group_order=rev)

            # ---- pvt spmm + W2/b2 + per-tile log_softmax ----
            def pvt_consume(t, ps, a):
                hb = spool.tile([NHID, P], bf16, tag="hT")
                nc.vector.tensor_tensor(out=hb[:], in0=ps[:], in1=a,
                                        op=mybir.AluOpType.add)
                ps2 = ps2pool.tile([P, NCLASS], f32, tag="ps2")
                mm1 = nc.tensor.matmul(out=ps2[:], lhsT=hb[:], rhs=w2_sb[:],
                                       start=True, stop=False)
                mm2 = nc.tensor.matmul(
                    out=ps2[:], lhsT=s_sb[0:1, t * P:(t + 1) * P],
                    rhs=b2_sb[:], start=False, stop=True)
                tile.add_dep_helper(mm2.ins, mm1.ins, sync=False,
                                    reason="acc order")
                mxt = fxpool.tile([P, 1], f32, tag="mxt")
                nc.vector.tensor_reduce(out=mxt[:], in_=ps2[:],
                                        axis=mybir.AxisListType.X,
                                        op=mybir.AluOpType.max)
                sh = fxpool.tile([P, NCLASS], f32, tag="sh")
                nc.vector.tensor_scalar(
                    out=sh[:], in0=ps2[:], scalar1=mxt[:, 0:1], scalar2=None,
                    op0=mybir.AluOpType.subtract)
                eb = fxpool.tile([P, NCLASS], f32, tag="eb")
                st = fxpool.tile([P, 1], f32, tag="st")
                nc.scalar.activation(out=eb[:], in_=sh[:],
                                     func=mybir.ActivationFunctionType.Exp,
                                     accum_out=st[:, 0:1])
                lst = fxpool.tile([P, 1], f32, tag="lst")
                nc.scalar.activation(out=lst[:], in_=st[:],
                                     func=mybir.ActivationFunctionType.Ln)
                ob = fxpool.tile([P, NCLASS], f32, tag="ob")
                nc.vector.tensor_scalar(
                    out=ob[:], in0=sh[:], scalar1=lst[:, 0:1], scalar2=None,
                    op0=mybir.AluOpType.subtract)
                nc.sync.dma_start(out=out_d.ap()[t * P:(t + 1) * P, :],
                                  in_=ob[:])

            # pvt: h=1 phase first (tab3-h1 staged first by gc2)
            spmm(pp, tab3, pidx_sb, plhs_d, pvt_consume, halves=(1, 0))

    nc.compile()
    return nc


# --------------------------------------------------------------------------
# host driver
# --------------------------------------------------------------------------

def _plan(inputs, G_T_adj=5, G_T_pvt=10):
    ep = Plan(np.asarray(inputs["adj_row"]).astype(np.int64),
              np.asarray(inputs["adj_col"]).astype(np.int64),
              np.asarray(inputs["adj_val"], np.float32), G_T_adj)
    pp = Plan(np.asarray(inputs["pvt_row"]).astype(np.int64),
              np.asarray(inputs["pvt_col"]).astype(np.int64),
              np.asarray(inputs["pvt_val"], np.float32), G_T_pvt)
    return ep, pp


def _run(inputs, trace=True, plans=None):
    NFEAT, NHID, NCLASS = FULL["NFEAT"], FULL["NHID"], FULL["NCLASS"]
    N = FULL["N"]
    if plans is None:
        plans = _plan(inputs)
    ep, pp = plans

    nc = build_kernel(ep, pp)

    x_pad = np.zeros((N_PAD, NFEAT), BF16)
    x_pad[:N] = np.asarray(inputs["x"], np.float32).astype(BF16)
    xt_full = np.ascontiguousarray(x_pad.T)  # [NFEAT, N_PAD]
    w1 = np.asarray(inputs["W1"], np.float32).astype(BF16)
    w2 = np.asarray(inputs["W2"], np.float32).astype(BF16)
    b1 = np.asarray(inputs["b1"], np.float32).reshape(NHID, 1)
    b2r = np.asarray(inputs["b2"], np.float32).reshape(1, NCLASS).astype(BF16)

    s_full = np.zeros(N_PAD, np.float32)
    np.add.at(s_full, np.asarray(inputs["pvt_row"]).astype(np.int64),
              np.asarray(inputs["pvt_val"], np.float32))

    in_maps = []
    for k in range(NCORES):
        in_maps.append({
            "xt": xt_full[:, k * N_LOC:(k + 1) * N_LOC],
            "w1": w1, "w2": w2, "b1": b1, "b2r": b2r,
            "srow": s_full[k * N_LOC:(k + 1) * N_LOC]
                    .reshape(1, N_LOC).astype(BF16),
            "eidx": ep.idx_np[k], "elhs": ep.lhs_np[k],
            "pidx": pp.idx_np[k], "plhs": pp.lhs_np[k],
        })

    res = run_bass_kernel_spmd(nc, in_maps, core_ids=list(range(NCORES)),
                               trace=trace)
    _run.last_exec_time_ns = res.exec_time_ns
    out = np.concatenate([r["out"] for r in res.results], axis=0)[:N]
    return np.ascontiguousarray(out.astype(np.float32))


_run.last_exec_time_ns = None


def kernel(**inputs) -> np.ndarray:
    return _run(inputs)
